# revision 21
# baseline (speedup 1.0000x reference)
"""Trainium2 Bass kernel for 16-head causal self-attention (KaplanAttention).

Problem: x [2, 2048, 1024], torch-style weights W_q/W_k/W_v/W_o [1024, 1024].
  q/k/v = (x @ W.T) split into 16 heads of 64; causal softmax(q k^T / 8) @ v;
  concat heads; out = attn_out @ W_o.T.

Sharding (8 cores): core c handles batch b = c // 4 and head group g = c % 4
(heads 4g..4g+3). Each core computes its 4 heads' attention output and a
partial output projection against the matching 256 columns of W_o; the host
sums the 4 partials per batch (the "all-reduce" of the row-sharded W_o).

Per-core layout (everything transposed on host so the PE contraction dim is
always the partition dim):
  xT  [1024, 2048] = x[b].T   fp16       e on partitions
  wqT/wkT/wvT [1024, 256]     fp16       e on partitions
  woT [256, 1024]             fp16       d on partitions
  QT/KT [128, 2, 2048] fp16: head pair hp, head h at partitions 64*(h%2)
  V     [128, 16, 4, 128] fp16: j-chunk k, head h -> [ones | 0*63 | V_h]
        (the ones col gives the softmax denominator for free from the same
        matmul; it sits first so Z lands in PSUM partition 0 where the
        buggy reciprocal_approx_fast reads, and V_h starts at column 64 so
        the value rows land at PSUM partition 64 -- a 64-partition DVE PSUM
        access may only start at partition 0 or 64)
  scores computed transposed: S^T[j, s] fp16; exp on ACT (scale=1/8 fused);
  causal handled by only computing s >= 128*jt and a {0,1} upper-tri mask
  on diagonal 128x128 blocks (one DVE mul per (hp,jt), both heads at once).
  U^T is stored in four per-512-s-block tiles (block p holds rows jt<=4p+3,
  cols [max(128jt,512p), 512(p+1))) so block p can be recycled for the next
  head pair as soon as this pair's AV for t=p has consumed it.
  AV: psum [128, s] accumulates [1|0|V]^T @ U^T; row 0 = Z_s. Normalize:
  reciprocal_approx_fast on psa[0:1] (the custom DVE op mis-addresses PSUM
  partition offsets -- it always reads partition 0, which is exactly where
  the ones-first layout puts Z), gpsimd partition_broadcast, tensor_mul
  -> outTn fp16.
  Final: partial[s, m] from lhsT = normalized out^T, rhs = woT chunks;
  partials are written fp16 and summed on host in fp32.

(fp8e4 + DoubleRow on the projection GEMMs was tried and is ~1.2x faster
end-to-end, but e4m3's ~6% per-element quantization lands almost fully on
the output — relative errors of zero-mean dot products do not average down
— and measured 7.4e-2 rel L2, over the 2e-2 bar.  Everything stays fp16.)

Scheduling: ACT (exp over ~4.5M score elements/core) and the PE are both
near-saturated; emission order keeps both streaming:
  - DMAs round-robin over the sync and gpsimd queues (a single queue
    serializes at ~600ns each); xT lands in 512-column blocks so block-0
    Q/K tiles unblock early;
  - scores are emitted s-block-major, with the Q/K projection tiles for
    block p emitted just before the block's score chunks;
  - score PSUM = 2 x [128, 2, 512] (4 banks) so the next chunk's matmuls
    run under the current chunk's exp -> ACT ~100% duty;
  - one shared 4-slot 1-bank PSUM pool rotates proj/AV/final tiles;
  - V projection and hp1's Q/K tiles are emitted after hp0's scores as PE
    fillers during hp0's exp stream.
"""

import numpy as np

from concourse import bass_utils, mybir, tile
from concourse import bacc

S = 2048
D = 1024
HPC = 4        # heads per core
DK = 64
DC = HPC * DK  # 256 d-columns per core
NCORES = 8
EC = D // 128  # 8 e-chunks
NJT = S // 128  # 16 j-tiles
NST = S // 512  # 4 s 512-tiles

FP16 = mybir.dt.float16
FP32 = mybir.dt.float32

# UT block p holds, for each row-tile jt <= 4p+3, the columns
# [max(128jt, 512p), 512(p+1)).  WIDTH[p][jt] is that width and BOFF[p][jt]
# the row's offset inside the block tile.
WIDTH = [[512 * (p + 1) - max(128 * jt, 512 * p) for jt in range(4 * p + 4)]
         for p in range(NST)]
BOFF = []
for p in range(NST):
    offs, o = [], 0
    for w in WIDTH[p]:
        offs.append(o)
        o += w
    BOFF.append(offs)
WTOT = [sum(ws) for ws in WIDTH]


def _build(reps=1):
    nc = bacc.Bacc("TRN2", target_bir_lowering=False, debug=False)

    xT_d = nc.dram_tensor("xT", [D, S], FP16, kind="ExternalInput")
    wq_d = nc.dram_tensor("wqT", [D, DC], FP16, kind="ExternalInput")
    wk_d = nc.dram_tensor("wkT", [D, DC], FP16, kind="ExternalInput")
    wv_d = nc.dram_tensor("wvT", [D, DC], FP16, kind="ExternalInput")
    wo_d = nc.dram_tensor("woT", [DC, D], FP16, kind="ExternalInput")
    mask_d = nc.dram_tensor("mask", [128, 2, 128], FP16, kind="ExternalInput")
    out_d = nc.dram_tensor("out", [S, D], FP16, kind="ExternalOutput")

    with tile.TileContext(nc) as tc:
        with (
            tc.tile_pool(name="const", bufs=1) as const,
            tc.tile_pool(name="work", bufs=1) as work,
            tc.tile_pool(name="ut", bufs=1) as utp,
            tc.tile_pool(name="outs", bufs=4) as outs,
            tc.tile_pool(name="norm", bufs=4) as normp,
            tc.tile_pool(name="ps1", bufs=4, space="PSUM") as ps1,
            tc.tile_pool(name="psS", bufs=2, space="PSUM") as psS,
        ):
          for _rep in range(reps):
            dmaq = [nc.sync, nc.gpsimd]
            _qi = [0]

            def dma(out, in_):
                dmaq[_qi[0] % len(dmaq)].dma_start(out=out, in_=in_)
                _qi[0] += 1

            wq = const.tile([128, EC, DC], FP16)
            wk = const.tile([128, EC, DC], FP16)
            for w_t, w_dr in ((wq, wq_d), (wk, wk_d)):
                dma(w_t, w_dr.rearrange("(c p) d -> p c d", p=128))
            mask = const.tile([128, 2, 128], FP16)
            dma(mask, mask_d[:, :, :])
            xT = const.tile([128, EC, S], FP16)
            for st in range(NST):
                for c in range(EC):
                    dma(
                        xT[:, c, 512 * st : 512 * (st + 1)],
                        xT_d[128 * c : 128 * (c + 1), 512 * st : 512 * (st + 1)],
                    )
            wv = const.tile([128, EC, DC], FP16)
            dma(wv, wv_d.rearrange("(c p) d -> p c d", p=128))
            wo = const.tile([128, 2, D], FP16)
            dma(wo, wo_d.rearrange("(c p) d -> p c d", p=128))

            QT = work.tile([128, 2, S], FP16)
            KT = work.tile([128, 2, S], FP16)

            def qk_proj(hp, st_list):
                for st in st_list:
                    for w_t, dst in ((wq, QT), (wk, KT)):
                        ps = ps1.tile([128, 512], FP32, tag="b1")
                        for c in range(EC):
                            nc.tensor.matmul(
                                ps,
                                w_t[:, c, 128 * hp : 128 * (hp + 1)],
                                xT[:, c, 512 * st : 512 * (st + 1)],
                                start=(c == 0),
                                stop=(c == EC - 1),
                            )
                        nc.vector.tensor_copy(
                            out=dst[:, hp, 512 * st : 512 * (st + 1)], in_=ps
                        )

            # V tile is filled by the deferred V projection below; the ones
            # column is set once up front.
            V = work.tile([128, NJT, HPC, 128], FP16)
            nc.vector.memset(V[:, :, :, 0:1], 1.0)
            nc.vector.memset(V[:, :, :, 1:64], 0.0)

            outTn = work.tile([128, 2, S], FP16)  # normalized out^T, pair-stacked

            def scores_block(hp, UTb, p):
                for jt in range(4 * p + 4):
                    s0 = 128 * jt
                    pos = max(s0, 512 * p)
                    cn = WIDTH[p][jt]
                    ps = psS.tile([128, 2, 512], FP32, tag="score")
                    for hi in range(2):
                        ho = 64 * hi
                        nc.tensor.matmul(
                            ps[:, hi, 0:cn],
                            KT[ho : ho + 64, hp, s0 : s0 + 128],
                            QT[ho : ho + 64, hp, pos : pos + cn],
                            start=True,
                            stop=True,
                        )
                    uo = BOFF[p][jt]
                    nc.scalar.activation(
                        out=UTb[p][:, :, uo : uo + cn],
                        in_=ps[:, :, 0:cn],
                        func=mybir.ActivationFunctionType.Exp,
                        scale=0.125,
                    )
                    if p == jt // 4:
                        # causal mask on the diagonal 128-block
                        nc.vector.tensor_mul(
                            UTb[p][:, :, uo : uo + 128],
                            UTb[p][:, :, uo : uo + 128],
                            mask,
                        )

            def scores(hp, UTb, with_proj):
                for p in range(NST):
                    if with_proj:
                        qk_proj(hp, [p])
                    scores_block(hp, UTb, p)

            def av_t(hp, UTb, t):
                for hi in range(2):
                    h = 2 * hp + hi
                    ho = 64 * hi
                    psa = ps1.tile([128, 512], FP32, tag="b1")
                    kmax = 4 * t + 4
                    for k in range(kmax):
                        off = max(0, 128 * k - 512 * t)
                        n = 512 - off
                        uo = BOFF[t][k]
                        nc.tensor.matmul(
                            psa[:, off : off + n],
                            V[:, k, h, :],
                            UTb[t][:, hi, uo : uo + n],
                            start=(k == 0),
                            stop=(k == kmax - 1),
                        )
                    zr = normp.tile([1, 512], FP32, tag="zrow")
                    nc.vector.reciprocal_approx_fast(out=zr, in_=psa[0:1, :])
                    zb = normp.tile([64, 512], FP32, tag="zb")
                    nc.gpsimd.partition_broadcast(zb, zr)
                    nc.vector.tensor_mul(
                        outTn[ho : ho + 64, hp, 512 * t : 512 * (t + 1)],
                        psa[64:128, :],
                        zb,
                    )

            def v_proj(jt_list):
                for jt in jt_list:
                    psv = ps1.tile([128, 512], FP32, tag="b1")
                    psd = psv[:, 0:DC]
                    for c in range(EC):
                        nc.tensor.matmul(
                            psd,
                            xT[:, c, 128 * jt : 128 * (jt + 1)],
                            wv[:, c, :],
                            start=(c == 0),
                            stop=(c == EC - 1),
                        )
                    nc.vector.tensor_copy(
                        out=V[:, jt, :, 64:128],
                        in_=psd.rearrange("p (h d) -> p h d", h=HPC),
                    )

            def final(st_list):
                for st in st_list:
                    for mt in range(2):
                        psf = ps1.tile([128, 512], FP32, tag="b1")
                        for hp in range(2):
                            nc.tensor.matmul(
                                psf,
                                outTn[:, hp, 128 * st : 128 * (st + 1)],
                                wo[:, hp, 512 * mt : 512 * (mt + 1)],
                                start=(hp == 0),
                                stop=(hp == 1),
                            )
                        ob = outs.tile([128, 512], FP16, tag="ob")
                        if mt == 0:
                            nc.vector.tensor_copy(out=ob, in_=psf)
                            eng = nc.sync
                        else:
                            nc.scalar.copy(out=ob, in_=psf)
                            eng = nc.scalar
                        eng.dma_start(
                            out=out_d[
                                128 * st : 128 * (st + 1), 512 * mt : 512 * (mt + 1)
                            ],
                            in_=ob,
                        )

            def ut_blocks(sfx):
                tiles = []
                for p in range(NST):
                    ub = utp.tile(
                        [128, 2, WTOT[p]], FP16, tag=f"ut{p}", name=f"ut{p}{sfx}"
                    )
                    tiles.append(ub)
                return tiles

            # ---- hp0 pipeline ----
            UT0 = ut_blocks("a")
            scores(0, UT0, with_proj=True)
            # hp1's score matmuls must outrank the fillers and hp0's AV on
            # the PE: otherwise, at the moment hp0's last exp lands, the PE
            # drains av0-t3 plus filler leftovers (~25us) before feeding ACT
            # its first hp1 chunk.  Leave a priority gap and emit hp1's
            # scores into it.  hp1's blocks 0-1 additionally get DEDICATED
            # UT tiles (blocks 2-3 reuse hp0's slots): with shared tags
            # their tiles could only be allocated after av0 was emitted,
            # which made the static scheduler queue ~350 lower-priority
            # matmuls ahead of them (a 14us ACT stall at the boundary).
            prio_scores1 = tc.cur_priority
            tc.cur_priority = prio_scores1 + 500
            qk_proj(1, range(NST))     # qk1 first — it gates hp1's scores
            UT1 = [
                utp.tile([128, 2, WTOT[0]], FP16, tag="ut0b", name="ut0b"),
                utp.tile([128, 2, WTOT[1]], FP16, tag="ut1b", name="ut1b"),
                None,
                None,
            ]
            with tc.high_priority(tc.cur_priority - prio_scores1):
                scores_block(1, UT1, 0)
                scores_block(1, UT1, 1)
            v_proj(range(NJT))
            for t in range(NST):
                av_t(0, UT0, t)

            # ---- hp1 tail: blocks 2-3 reuse hp0's UT slots (freed by
            # av0-t2/t3) ----
            UT1[2] = utp.tile([128, 2, WTOT[2]], FP16, tag="ut2", name="ut2b")
            UT1[3] = utp.tile([128, 2, WTOT[3]], FP16, tag="ut3", name="ut3b")
            with tc.high_priority(tc.cur_priority - prio_scores1 + 100):
                scores_block(1, UT1, 2)
                scores_block(1, UT1, 3)
            for t in range(NST):
                av_t(1, UT1, t)

            # ---- final projection ----
            final(range(NJT))

    nc.compile()
    return nc


_NC = None


def _prep_in_maps(x, W_q, W_k, W_v, W_o):
    x = np.asarray(x, dtype=np.float32)
    W_q = np.asarray(W_q, dtype=np.float32)
    W_k = np.asarray(W_k, dtype=np.float32)
    W_v = np.asarray(W_v, dtype=np.float32)
    W_o = np.asarray(W_o, dtype=np.float32)
    mask01 = np.triu(np.ones((128, 128), dtype=np.float16))
    mask2 = np.ascontiguousarray(
        np.broadcast_to(mask01[:, None, :], (128, 2, 128))
    )
    in_maps = []
    for c in range(NCORES):
        b, g = divmod(c, 4)
        cols = slice(DC * g, DC * (g + 1))
        in_maps.append(
            {
                "xT": np.ascontiguousarray(x[b].T).astype(np.float16),
                "wqT": np.ascontiguousarray(W_q[cols, :].T).astype(np.float16),
                "wkT": np.ascontiguousarray(W_k[cols, :].T).astype(np.float16),
                "wvT": np.ascontiguousarray(W_v[cols, :].T).astype(np.float16),
                "woT": np.ascontiguousarray(W_o[:, cols].T).astype(np.float16),
                "mask": mask2,
            }
        )
    return in_maps


def _run(x, W_q, W_k, W_v, W_o, **spmd_kwargs):
    global _NC
    if _NC is None:
        _NC = _build()
    in_maps = _prep_in_maps(x, W_q, W_k, W_v, W_o)
    res = bass_utils.run_bass_kernel_spmd(
        _NC, in_maps, core_ids=list(range(NCORES)), **spmd_kwargs
    )
    out = np.empty((2, S, D), dtype=np.float32)
    for b in range(2):
        acc = res.results[4 * b]["out"].astype(np.float32)
        for g in range(1, 4):
            acc += res.results[4 * b + g]["out"].astype(np.float32)
        out[b] = acc
    return out, res


def kernel(x, W_q, W_k, W_v, W_o):
    out, _ = _run(x, W_q, W_k, W_v, W_o)
    return out


# revision 22
# speedup vs baseline: 1.0248x; 1.0248x over previous
"""Trainium2 Bass kernel for 16-head causal self-attention (KaplanAttention).

Problem: x [2, 2048, 1024], torch-style weights W_q/W_k/W_v/W_o [1024, 1024].
  q/k/v = (x @ W.T) split into 16 heads of 64; causal softmax(q k^T / 8) @ v;
  concat heads; out = attn_out @ W_o.T.

Sharding (8 cores): core c handles batch b = c // 4 and head group g = c % 4
(heads 4g..4g+3). Each core computes its 4 heads' attention output and a
partial output projection against the matching 256 columns of W_o; the host
sums the 4 partials per batch (the "all-reduce" of the row-sharded W_o).

Per-core layout (everything transposed on host so the PE contraction dim is
always the partition dim):
  xT  [1024, 2048] = x[b].T   fp16       e on partitions
  wqT/wkT/wvT [1024, 256]     fp16       e on partitions
  woT [256, 1024]             fp16       d on partitions
  QT/KT [128, 2, 2048] fp16: head pair hp, head h at partitions 64*(h%2)
  V     [128, 16, 4, 128] fp16: j-chunk k, head h -> [ones | 0*63 | V_h]
        (the ones col gives the softmax denominator for free from the same
        matmul; it sits first so Z lands in PSUM partition 0 where the
        buggy reciprocal_approx_fast reads, and V_h starts at column 64 so
        the value rows land at PSUM partition 64 -- a 64-partition DVE PSUM
        access may only start at partition 0 or 64)
  scores computed transposed: S^T[j, s] fp16; exp on ACT (scale=1/8 fused);
  causal handled by only computing s >= 128*jt and a {0,1} upper-tri mask
  on diagonal 128x128 blocks (one DVE mul per (hp,jt), both heads at once).
  U^T is stored in four per-512-s-block tiles (block p holds rows jt<=4p+3,
  cols [max(128jt,512p), 512(p+1))) so block p can be recycled for the next
  head pair as soon as this pair's AV for t=p has consumed it.
  AV: psum [128, s] accumulates [1|0|V]^T @ U^T; row 0 = Z_s. Normalize:
  reciprocal_approx_fast on psa[0:1] (the custom DVE op mis-addresses PSUM
  partition offsets -- it always reads partition 0, which is exactly where
  the ones-first layout puts Z), gpsimd partition_broadcast, tensor_mul
  -> outTn fp16.
  Final: partial[s, m] from lhsT = normalized out^T, rhs = woT chunks;
  partials are written fp16 and summed on host in fp32.

(fp8e4 + DoubleRow on the projection GEMMs was tried and is ~1.2x faster
end-to-end, but e4m3's ~6% per-element quantization lands almost fully on
the output — relative errors of zero-mean dot products do not average down
— and measured 7.4e-2 rel L2, over the 2e-2 bar.  Everything stays fp16.)

Scheduling: ACT (exp over ~4.5M score elements/core) and the PE are both
near-saturated; emission order keeps both streaming:
  - DMAs round-robin over the sync and gpsimd queues (a single queue
    serializes at ~600ns each); xT lands in 512-column blocks so block-0
    Q/K tiles unblock early;
  - scores are emitted s-block-major, with the Q/K projection tiles for
    block p emitted just before the block's score chunks;
  - score PSUM = 2 x [128, 2, 512] (4 banks) so the next chunk's matmuls
    run under the current chunk's exp -> ACT ~100% duty;
  - one shared 4-slot 1-bank PSUM pool rotates proj/AV/final tiles;
  - V projection and hp1's Q/K tiles are emitted after hp0's scores as PE
    fillers during hp0's exp stream.
"""

import numpy as np
import ml_dtypes

from concourse import bass_utils, mybir, tile
from concourse import bacc

S = 2048
D = 1024
HPC = 4        # heads per core
DK = 64
DC = HPC * DK  # 256 d-columns per core
NCORES = 8
EC = D // 128  # 8 e-chunks
NJT = S // 128  # 16 j-tiles
NST = S // 512  # 4 s 512-tiles

FP16 = mybir.dt.bfloat16  # bf16: moving operand streams 2/cycle on HW
FP32 = mybir.dt.float32

# UT block p holds, for each row-tile jt <= 4p+3, the columns
# [max(128jt, 512p), 512(p+1)).  WIDTH[p][jt] is that width and BOFF[p][jt]
# the row's offset inside the block tile.
WIDTH = [[512 * (p + 1) - max(128 * jt, 512 * p) for jt in range(4 * p + 4)]
         for p in range(NST)]
BOFF = []
for p in range(NST):
    offs, o = [], 0
    for w in WIDTH[p]:
        offs.append(o)
        o += w
    BOFF.append(offs)
WTOT = [sum(ws) for ws in WIDTH]


def _build(reps=1):
    nc = bacc.Bacc("TRN2", target_bir_lowering=False, debug=False)

    xT_d = nc.dram_tensor("xT", [D, S], FP16, kind="ExternalInput")
    wq_d = nc.dram_tensor("wqT", [D, DC], FP16, kind="ExternalInput")
    wk_d = nc.dram_tensor("wkT", [D, DC], FP16, kind="ExternalInput")
    wv_d = nc.dram_tensor("wvT", [D, DC], FP16, kind="ExternalInput")
    wo_d = nc.dram_tensor("woT", [DC, D], FP16, kind="ExternalInput")
    mask_d = nc.dram_tensor("mask", [128, 2, 128], FP16, kind="ExternalInput")
    out_d = nc.dram_tensor("out", [S, D], FP16, kind="ExternalOutput")

    with tile.TileContext(nc) as tc:
        with (
            tc.tile_pool(name="const", bufs=1) as const,
            tc.tile_pool(name="work", bufs=1) as work,
            tc.tile_pool(name="ut", bufs=1) as utp,
            tc.tile_pool(name="outs", bufs=4) as outs,
            tc.tile_pool(name="norm", bufs=4) as normp,
            tc.tile_pool(name="ps1", bufs=4, space="PSUM") as ps1,
            tc.tile_pool(name="psS", bufs=2, space="PSUM") as psS,
        ):
          for _rep in range(reps):
            dmaq = [nc.sync, nc.gpsimd]
            _qi = [0]

            def dma(out, in_):
                dmaq[_qi[0] % len(dmaq)].dma_start(out=out, in_=in_)
                _qi[0] += 1

            wq = const.tile([128, EC, DC], FP16)
            wk = const.tile([128, EC, DC], FP16)
            for w_t, w_dr in ((wq, wq_d), (wk, wk_d)):
                dma(w_t, w_dr.rearrange("(c p) d -> p c d", p=128))
            mask = const.tile([128, 2, 128], FP16)
            dma(mask, mask_d[:, :, :])
            xT = const.tile([128, EC, S], FP16)
            for st in range(NST):
                for c in range(EC):
                    dma(
                        xT[:, c, 512 * st : 512 * (st + 1)],
                        xT_d[128 * c : 128 * (c + 1), 512 * st : 512 * (st + 1)],
                    )
            wv = const.tile([128, EC, DC], FP16)
            dma(wv, wv_d.rearrange("(c p) d -> p c d", p=128))
            wo = const.tile([128, 2, D], FP16)
            dma(wo, wo_d.rearrange("(c p) d -> p c d", p=128))

            QT = work.tile([128, 2, S], FP16)
            KT = work.tile([128, 2, S], FP16)

            def qk_proj(hp, st_list):
                for st in st_list:
                    for w_t, dst in ((wq, QT), (wk, KT)):
                        ps = ps1.tile([128, 512], FP32, tag="b1")
                        for c in range(EC):
                            nc.tensor.matmul(
                                ps,
                                w_t[:, c, 128 * hp : 128 * (hp + 1)],
                                xT[:, c, 512 * st : 512 * (st + 1)],
                                start=(c == 0),
                                stop=(c == EC - 1),
                            )
                        nc.vector.tensor_copy(
                            out=dst[:, hp, 512 * st : 512 * (st + 1)], in_=ps
                        )

            # V tile is filled by the deferred V projection below; the ones
            # column is set once up front.
            V = work.tile([128, NJT, HPC, 128], FP16)
            nc.vector.memset(V[:, :, :, 0:1], 1.0)
            nc.vector.memset(V[:, :, :, 1:64], 0.0)

            outTn = work.tile([128, 2, S], FP16)  # normalized out^T, pair-stacked

            def scores_block(hp, UTb, p):
                for jt in range(4 * p + 4):
                    s0 = 128 * jt
                    pos = max(s0, 512 * p)
                    cn = WIDTH[p][jt]
                    ps = psS.tile([128, 2, 512], FP32, tag="score")
                    for hi in range(2):
                        ho = 64 * hi
                        nc.tensor.matmul(
                            ps[:, hi, 0:cn],
                            KT[ho : ho + 64, hp, s0 : s0 + 128],
                            QT[ho : ho + 64, hp, pos : pos + cn],
                            start=True,
                            stop=True,
                        )
                    uo = BOFF[p][jt]
                    nc.scalar.activation(
                        out=UTb[p][:, :, uo : uo + cn],
                        in_=ps[:, :, 0:cn],
                        func=mybir.ActivationFunctionType.Exp,
                        scale=0.125,
                    )
                    if p == jt // 4:
                        # causal mask on the diagonal 128-block
                        nc.vector.tensor_mul(
                            UTb[p][:, :, uo : uo + 128],
                            UTb[p][:, :, uo : uo + 128],
                            mask,
                        )

            def scores(hp, UTb, with_proj):
                for p in range(NST):
                    if with_proj:
                        qk_proj(hp, [p])
                    scores_block(hp, UTb, p)

            def av_t(hp, UTb, t):
                for hi in range(2):
                    h = 2 * hp + hi
                    ho = 64 * hi
                    psa = ps1.tile([128, 512], FP32, tag="b1")
                    kmax = 4 * t + 4
                    for k in range(kmax):
                        off = max(0, 128 * k - 512 * t)
                        n = 512 - off
                        uo = BOFF[t][k]
                        nc.tensor.matmul(
                            psa[:, off : off + n],
                            V[:, k, h, :],
                            UTb[t][:, hi, uo : uo + n],
                            start=(k == 0),
                            stop=(k == kmax - 1),
                        )
                    zr = normp.tile([1, 512], FP32, tag="zrow")
                    nc.vector.reciprocal_approx_fast(out=zr, in_=psa[0:1, :])
                    zb = normp.tile([64, 512], FP32, tag="zb")
                    nc.gpsimd.partition_broadcast(zb, zr)
                    nc.vector.tensor_mul(
                        outTn[ho : ho + 64, hp, 512 * t : 512 * (t + 1)],
                        psa[64:128, :],
                        zb,
                    )

            def v_proj(jt_list):
                for jt in jt_list:
                    psv = ps1.tile([128, 512], FP32, tag="b1")
                    psd = psv[:, 0:DC]
                    for c in range(EC):
                        nc.tensor.matmul(
                            psd,
                            xT[:, c, 128 * jt : 128 * (jt + 1)],
                            wv[:, c, :],
                            start=(c == 0),
                            stop=(c == EC - 1),
                        )
                    nc.vector.tensor_copy(
                        out=V[:, jt, :, 64:128],
                        in_=psd.rearrange("p (h d) -> p h d", h=HPC),
                    )

            def final(st_list):
                for st in st_list:
                    for mt in range(2):
                        psf = ps1.tile([128, 512], FP32, tag="b1")
                        for hp in range(2):
                            nc.tensor.matmul(
                                psf,
                                outTn[:, hp, 128 * st : 128 * (st + 1)],
                                wo[:, hp, 512 * mt : 512 * (mt + 1)],
                                start=(hp == 0),
                                stop=(hp == 1),
                            )
                        ob = outs.tile([128, 512], FP16, tag="ob")
                        if mt == 0:
                            nc.vector.tensor_copy(out=ob, in_=psf)
                            eng = nc.sync
                        else:
                            nc.scalar.copy(out=ob, in_=psf)
                            eng = nc.scalar
                        eng.dma_start(
                            out=out_d[
                                128 * st : 128 * (st + 1), 512 * mt : 512 * (mt + 1)
                            ],
                            in_=ob,
                        )

            def ut_blocks(sfx):
                tiles = []
                for p in range(NST):
                    ub = utp.tile(
                        [128, 2, WTOT[p]], FP16, tag=f"ut{p}", name=f"ut{p}{sfx}"
                    )
                    tiles.append(ub)
                return tiles

            # ---- hp0 pipeline ----
            UT0 = ut_blocks("a")
            scores(0, UT0, with_proj=True)
            # hp1's score matmuls must outrank the fillers and hp0's AV on
            # the PE: otherwise, at the moment hp0's last exp lands, the PE
            # drains av0-t3 plus filler leftovers (~25us) before feeding ACT
            # its first hp1 chunk.  Leave a priority gap and emit hp1's
            # scores into it.  hp1's blocks 0-1 additionally get DEDICATED
            # UT tiles (blocks 2-3 reuse hp0's slots): with shared tags
            # their tiles could only be allocated after av0 was emitted,
            # which made the static scheduler queue ~350 lower-priority
            # matmuls ahead of them (a 14us ACT stall at the boundary).
            prio_scores1 = tc.cur_priority
            tc.cur_priority = prio_scores1 + 500
            qk_proj(1, range(NST))     # qk1 first — it gates hp1's scores
            UT1 = [
                utp.tile([128, 2, WTOT[0]], FP16, tag="ut0b", name="ut0b"),
                utp.tile([128, 2, WTOT[1]], FP16, tag="ut1b", name="ut1b"),
                None,
                None,
            ]
            with tc.high_priority(tc.cur_priority - prio_scores1):
                scores_block(1, UT1, 0)
                scores_block(1, UT1, 1)
            v_proj(range(NJT))
            for t in range(NST):
                av_t(0, UT0, t)

            # ---- hp1 tail: blocks 2-3 reuse hp0's UT slots (freed by
            # av0-t2/t3) ----
            UT1[2] = utp.tile([128, 2, WTOT[2]], FP16, tag="ut2", name="ut2b")
            UT1[3] = utp.tile([128, 2, WTOT[3]], FP16, tag="ut3", name="ut3b")
            with tc.high_priority(tc.cur_priority - prio_scores1 + 100):
                scores_block(1, UT1, 2)
                scores_block(1, UT1, 3)
            for t in range(NST):
                av_t(1, UT1, t)

            # ---- final projection ----
            final(range(NJT))

    nc.compile()
    return nc


_NC = None


def _prep_in_maps(x, W_q, W_k, W_v, W_o):
    x = np.asarray(x, dtype=np.float32)
    W_q = np.asarray(W_q, dtype=np.float32)
    W_k = np.asarray(W_k, dtype=np.float32)
    W_v = np.asarray(W_v, dtype=np.float32)
    W_o = np.asarray(W_o, dtype=np.float32)
    mask01 = np.triu(np.ones((128, 128), dtype=ml_dtypes.bfloat16))
    mask2 = np.ascontiguousarray(
        np.broadcast_to(mask01[:, None, :], (128, 2, 128))
    )
    in_maps = []
    for c in range(NCORES):
        b, g = divmod(c, 4)
        cols = slice(DC * g, DC * (g + 1))
        in_maps.append(
            {
                "xT": np.ascontiguousarray(x[b].T).astype(ml_dtypes.bfloat16),
                "wqT": np.ascontiguousarray(W_q[cols, :].T).astype(ml_dtypes.bfloat16),
                "wkT": np.ascontiguousarray(W_k[cols, :].T).astype(ml_dtypes.bfloat16),
                "wvT": np.ascontiguousarray(W_v[cols, :].T).astype(ml_dtypes.bfloat16),
                "woT": np.ascontiguousarray(W_o[:, cols].T).astype(ml_dtypes.bfloat16),
                "mask": mask2,
            }
        )
    return in_maps


def _run(x, W_q, W_k, W_v, W_o, **spmd_kwargs):
    global _NC
    if _NC is None:
        _NC = _build()
    in_maps = _prep_in_maps(x, W_q, W_k, W_v, W_o)
    res = bass_utils.run_bass_kernel_spmd(
        _NC, in_maps, core_ids=list(range(NCORES)), **spmd_kwargs
    )
    out = np.empty((2, S, D), dtype=np.float32)
    for b in range(2):
        acc = res.results[4 * b]["out"].astype(np.float32)
        for g in range(1, 4):
            acc += res.results[4 * b + g]["out"].astype(np.float32)
        out[b] = acc
    return out, res


def kernel(x, W_q, W_k, W_v, W_o):
    out, _ = _run(x, W_q, W_k, W_v, W_o)
    return out
